# revision 11
# baseline (speedup 1.0000x reference)
"""IntraSentenceAttention Trainium2 kernel.

out[b,t,:] = sum_s P[t,s] x[b,s,:],  P = softmax-like row-normalized
             exp(x@x^T + min(t-s,10)) masked by m_t, m_s  (no max-subtract).

Strategy (8 NeuronCores, data-parallel over batch, 4 batches/core):
  - scores computed transposed, E^T[s,t], so the PV matmul needs no transpose:
      S^T tile = matmul(lhsT=xT[:,s_blk], rhs=xT[:,t_blk])   (f32r, full rate)
      E^T      = exp(S^T - 2) * expdistT[s,t]                (ACT exp + DVE mul)
    The -2 bias keeps values in fp16 range (max exp(10+score) ~ 9.9e4 > fp16max);
    a constant factor cancels in the row normalization.
  - dist bias applied multiplicatively: expdistT = exp(min(t-s,10)) precomputed
    host-side (fp16, resident in SBUF). Tiles with t_tile <= s_tile-2 are exactly
    zero (fp32/fp16 underflow) and are skipped everywhere: 43 of 64 tiles survive.
  - mask folded host-side into the PV moving operand: xm = [m_s*x | m_s] fp16.
    Column 128 of the PV accumulation then yields the row sum r_t for free.
  - epilogue per 128-row output tile: out = (acc * 1/(r+eps') * m_t), eps' = eps*e^-2.
"""

import sys

sys.path.insert(0, "/opt/trn_rl_repo")

import numpy as np

import concourse.bacc as bacc
import concourse.tile as tile
from concourse import mybir
from concourse.bass_utils import run_bass_kernel_spmd

B, T, D = 32, 1024, 128
NCORES = 8
BPC = B // NCORES          # batches per core
NT = T // 128              # 128-row tiles per sequence
DIST_CAP = 10
EXP_BIAS = -2.0            # folded into exp; cancels in normalization
EPS = 1e-7 * float(np.exp(EXP_BIAS))

F32 = mybir.dt.float32
F32R = mybir.dt.float32r
F16 = mybir.dt.float16


def _span(s):
    """Nonzero t-range start for s-tile row: tiles with t_tile <= s_tile-2 are 0."""
    return max(0, s - 1) * 128


def _chunks(w):
    """Split a score-row width into matmul chunks that never cross a PSUM bank
    (512 f32 cols) boundary. All chunks >=256 run f32r at full rate; the one
    128-wide tail (w==640) pays the 4x f32 rate on 128 cols only."""
    if w <= 512:
        return [w]
    if w == 640:
        return [512, 128]
    return [512, w - 512]


def _build_module():
    nc = bacc.Bacc("TRN2", target_bir_lowering=False, debug=False, num_devices=NCORES)
    xT = nc.declare_dram_parameter("xT", [BPC, D, T], F32R, isOutput=False)
    xm = nc.declare_dram_parameter("xm", [BPC, T, D + 1], F16, isOutput=False)
    ed = nc.declare_dram_parameter("ed", [NT, 128, T], F16, isOutput=False)
    mt = nc.declare_dram_parameter("mt", [BPC, NT, 128], F32, isOutput=False)
    y = nc.declare_dram_parameter("y", [BPC, T, D], F32, isOutput=True)

    Exp = mybir.ActivationFunctionType.Exp
    Mult = mybir.AluOpType.mult

    with tile.TileContext(nc) as tc:
        with (
            tc.tile_pool(name="edp", bufs=1) as edp,
            tc.tile_pool(name="xtp", bufs=2) as xtp,
            tc.tile_pool(name="xmp", bufs=2) as xmp,
            tc.tile_pool(name="ep", bufs=2) as epool,
            tc.tile_pool(name="sm", bufs=8) as small,
            tc.tile_pool(name="op", bufs=4) as opool,
            tc.tile_pool(name="ps_s", bufs=3, space="PSUM") as ps_s,
            tc.tile_pool(name="ps_o", bufs=2, space="PSUM") as ps_o,
        ):
            bias_t = edp.tile([128, 1], F32, tag="bias")
            nc.vector.memset(bias_t, EXP_BIAS)

            # resident distance-bias tiles: one DMA, sliced per s-row
            edt = edp.tile([128, NT, T], F16, tag="ed")
            nc.sync.dma_start(out=edt, in_=ed.rearrange("nt p t -> p nt t"))
            ed_tiles = [edt[:, s, _span(s) : T] for s in range(NT)]

            state = {}

            def emit_scores(b):
                xTt = xtp.tile([128, T], F32R, tag="xT")
                nc.sync.dma_start(out=xTt, in_=xT[b])
                # one strided DMA each for the PV moving operand and the t-mask
                xmt = xmp.tile([128, NT, D + 1], F16, tag="xm")
                nc.sync.dma_start(
                    out=xmt, in_=xm[b].rearrange("(nt p) c -> p nt c", p=128)
                )
                mtile = xmp.tile([128, NT], F32, tag="mtile")
                nc.sync.dma_start(out=mtile, in_=mt[b].rearrange("nt p -> p nt"))
                Es = []
                for s in range(NT):
                    t0 = _span(s)
                    w = T - t0
                    pS = ps_s.tile([128, w], F32, tag="pscore")
                    off = 0
                    for cw in _chunks(w):
                        nc.tensor.matmul(
                            pS[:, off : off + cw],
                            lhsT=xTt[:, s * 128 : (s + 1) * 128],
                            rhs=xTt[:, t0 + off : t0 + off + cw],
                            start=True,
                            stop=True,
                        )
                        off += cw
                    Et = epool.tile([128, w], F16, tag=f"E{s}")
                    nc.scalar.activation(out=Et, in_=pS, func=Exp, bias=bias_t, scale=1.0)
                    nc.vector.tensor_mul(Et, Et, ed_tiles[s])
                    Es.append(Et)
                state[b] = (xmt, mtile, Es)

            def emit_pv(b):
                xmt, mtile, Es = state.pop(b)
                obig = opool.tile([128, NT, D], F32, tag="out")
                for tt in range(NT):
                    pO = ps_o.tile([128, D + 1], F32, tag="pout")
                    s_list = list(range(min(NT, tt + 2)))
                    for i, s in enumerate(s_list):
                        sl = tt * 128 - _span(s)
                        nc.tensor.matmul(
                            pO,
                            lhsT=Es[s][:, sl : sl + 128],
                            rhs=xmt[:, s, :],
                            start=(i == 0),
                            stop=(i == len(s_list) - 1),
                        )
                    rr = small.tile([128, 1], F32, tag="r")
                    nc.vector.tensor_scalar_add(rr, pO[:, D : D + 1], EPS)
                    nc.vector.reciprocal(rr, rr)
                    nc.vector.tensor_scalar(
                        out=obig[:, tt, :],
                        in0=pO[:, 0:D],
                        scalar1=rr,
                        scalar2=mtile[:, tt : tt + 1],
                        op0=Mult,
                        op1=Mult,
                    )
                nc.sync.dma_start(
                    out=y[b].rearrange("(nt p) d -> p nt d", p=128), in_=obig
                )

            # software pipeline: scores one batch ahead of PV
            for b in range(BPC):
                emit_scores(b)
                if b > 0:
                    emit_pv(b - 1)
            emit_pv(BPC - 1)

    nc.compile()
    return nc


_NC = None


def _get_module():
    global _NC
    if _NC is None:
        _NC = _build_module()
    return _NC


def prepare_inputs(x, mask):
    """Host-side prep: per-core input dicts (cheap O(B*T*D) / O(T^2) work)."""
    x = np.asarray(x, dtype=np.float32)
    m = np.asarray(mask).astype(np.float32)
    idx = np.arange(T)
    dist = np.minimum(idx[None, :] - idx[:, None], DIST_CAP).astype(np.float64)
    edT = np.exp(dist).astype(np.float16)  # [s, t]
    ed = np.ascontiguousarray(edT.reshape(NT, 128, T))
    xm_full = np.concatenate([x * m[:, :, None], m[:, :, None]], axis=2).astype(np.float16)
    xT_full = np.ascontiguousarray(x.transpose(0, 2, 1))
    mt_full = np.ascontiguousarray(m.reshape(B // BPC, BPC, NT, 128))
    in_maps = []
    for c in range(NCORES):
        sl = slice(c * BPC, (c + 1) * BPC)
        in_maps.append(
            {
                "xT": np.ascontiguousarray(xT_full[sl]),
                "xm": np.ascontiguousarray(xm_full[sl]),
                "ed": ed,
                "mt": mt_full[c],
            }
        )
    return in_maps


def kernel(x, mask):
    nc = _get_module()
    in_maps = prepare_inputs(x, mask)
    res = run_bass_kernel_spmd(nc, in_maps, core_ids=list(range(NCORES)))
    out = np.concatenate([res.results[c]["y"] for c in range(NCORES)], axis=0)
    return out.reshape(B, T, D).astype(np.float32)


# revision 13
# speedup vs baseline: 1.4266x; 1.4266x over previous
"""IntraSentenceAttention Trainium2 kernel.

out[b,t,:] = sum_s P[t,s] x[b,s,:],  P = softmax-like row-normalized
             exp(x@x^T + min(t-s,10)) masked by m_t, m_s  (no max-subtract).

Strategy (8 NeuronCores, data-parallel over batch, 4 batches/core):
  - scores computed transposed, E^T[s,t], so the PV matmul needs no transpose:
      S^T tile = matmul(lhsT=xT[:,s_blk], rhs=xT[:,t_blk])   (f32r, full rate)
      E^T      = exp(S^T - 2) * expdistT[s,t]                (ACT exp + DVE mul)
    The -2 bias keeps values in fp16 range (max exp(10+score) ~ 9.9e4 > fp16max);
    a constant factor cancels in the row normalization.
  - dist bias applied multiplicatively: expdistT = exp(min(t-s,10)) precomputed
    host-side (fp16, resident in SBUF). Tiles with t_tile <= s_tile-2 are exactly
    zero (fp32/fp16 underflow) and are skipped everywhere: 43 of 64 tiles survive.
  - mask folded host-side into the PV moving operand: xm = [m_s*x | m_s] fp16.
    Column 128 of the PV accumulation then yields the row sum r_t for free.
  - epilogue per 128-row output tile: out = (acc * 1/(r+eps') * m_t), eps' = eps*e^-2.
"""

import sys

sys.path.insert(0, "/opt/trn_rl_repo")

import numpy as np

import concourse.bacc as bacc
import concourse.tile as tile
from concourse import mybir
from concourse.bass_utils import run_bass_kernel_spmd

B, T, D = 32, 1024, 128
NCORES = 8
BPC = B // NCORES          # batches per core
NT = T // 128              # 128-row tiles per sequence
DIST_CAP = 10
EXP_BIAS = -2.0            # folded into exp; cancels in normalization
EPS = 1e-7 * float(np.exp(EXP_BIAS))

F32 = mybir.dt.float32
F32R = mybir.dt.float32r
F16 = mybir.dt.float16


def _span(s):
    """Nonzero t-range start for s-tile row: tiles with t_tile <= s_tile-2 are 0."""
    return max(0, s - 1) * 128


def _chunks(w):
    """Split a score-row width into matmul chunks that never cross a PSUM bank
    (512 f32 cols) boundary. All chunks >=256 run f32r at full rate; the one
    128-wide tail (w==640) pays the 4x f32 rate on 128 cols only."""
    if w <= 512:
        return [w]
    if w == 640:
        return [512, 128]
    return [512, w - 512]


def _build_module():
    nc = bacc.Bacc("TRN2", target_bir_lowering=False, debug=False, num_devices=NCORES)
    xT = nc.declare_dram_parameter("xT", [BPC, D, T], F32R, isOutput=False)
    xm = nc.declare_dram_parameter("xm", [BPC, T, D + 1], F16, isOutput=False)
    ed = nc.declare_dram_parameter("ed", [NT, 128, T], F16, isOutput=False)
    mt = nc.declare_dram_parameter("mt", [BPC, NT, 128], F32, isOutput=False)
    y = nc.declare_dram_parameter("y", [BPC, T, D], F32, isOutput=True)

    Exp = mybir.ActivationFunctionType.Exp
    Mult = mybir.AluOpType.mult

    with tile.TileContext(nc) as tc:
        with (
            tc.tile_pool(name="edp", bufs=1) as edp,
            tc.tile_pool(name="xtp", bufs=2) as xtp,
            tc.tile_pool(name="xmp", bufs=2) as xmp,
            tc.tile_pool(name="ep", bufs=2) as epool,
            tc.tile_pool(name="sm", bufs=8) as small,
            tc.tile_pool(name="op", bufs=4) as opool,
            tc.tile_pool(name="ps_s", bufs=3, space="PSUM") as ps_s,
            tc.tile_pool(name="ps_o", bufs=2, space="PSUM") as ps_o,
        ):
            bias_t = edp.tile([128, 1], F32, tag="bias")
            nc.vector.memset(bias_t, EXP_BIAS)

            # resident distance-bias tiles: one tile, loaded per s-row so the
            # first rows are usable before the whole 2MB lands
            edt = edp.tile([128, NT, T], F16, tag="ed")
            ed_tiles = [edt[:, s, _span(s) : T] for s in range(NT)]

            def emit_ed_loads():
                for s in range(NT):
                    t0 = _span(s)
                    nc.sync.dma_start(out=edt[:, s, t0:T], in_=ed[s, :, t0:T])

            state = {}
            loads = {}

            def emit_loads(b):
                xTt = xtp.tile([128, T], F32R, tag="xT")
                nc.sync.dma_start(out=xTt, in_=xT[b])
                # one strided DMA each for the PV moving operand and the t-mask
                xmt = xmp.tile([128, NT, D + 1], F16, tag="xm")
                nc.sync.dma_start(
                    out=xmt, in_=xm[b].rearrange("(nt p) c -> p nt c", p=128)
                )
                mtile = xmp.tile([128, NT], F32, tag="mtile")
                nc.sync.dma_start(out=mtile, in_=mt[b].rearrange("nt p -> p nt"))
                loads[b] = (xTt, xmt, mtile)

            def emit_scores(b):
                xTt, xmt, mtile = loads.pop(b)
                Es = []
                for s in range(NT):
                    t0 = _span(s)
                    w = T - t0
                    pS = ps_s.tile([128, w], F32, tag="pscore")
                    off = 0
                    for cw in _chunks(w):
                        nc.tensor.matmul(
                            pS[:, off : off + cw],
                            lhsT=xTt[:, s * 128 : (s + 1) * 128],
                            rhs=xTt[:, t0 + off : t0 + off + cw],
                            start=True,
                            stop=True,
                        )
                        off += cw
                    Et = epool.tile([128, w], F16, tag=f"E{s}")
                    nc.scalar.activation(out=Et, in_=pS, func=Exp, bias=bias_t, scale=1.0)
                    nc.vector.tensor_mul(Et, Et, ed_tiles[s])
                    Es.append(Et)
                state[b] = (xmt, mtile, Es)

            def emit_pv(b):
                xmt, mtile, Es = state.pop(b)
                obig = opool.tile([128, NT, D], F32, tag="out")
                for tt in range(NT):
                    pO = ps_o.tile([128, D + 1], F32, tag="pout")
                    s_list = list(range(min(NT, tt + 2)))
                    for i, s in enumerate(s_list):
                        sl = tt * 128 - _span(s)
                        nc.tensor.matmul(
                            pO,
                            lhsT=Es[s][:, sl : sl + 128],
                            rhs=xmt[:, s, :],
                            start=(i == 0),
                            stop=(i == len(s_list) - 1),
                        )
                    rr = small.tile([128, 1], F32, tag="r")
                    nc.vector.tensor_scalar_add(rr, pO[:, D : D + 1], EPS)
                    nc.vector.reciprocal(rr, rr)
                    nc.vector.tensor_scalar(
                        out=obig[:, tt, :],
                        in0=pO[:, 0:D],
                        scalar1=rr,
                        scalar2=mtile[:, tt : tt + 1],
                        op0=Mult,
                        op1=Mult,
                    )
                nc.sync.dma_start(
                    out=y[b].rearrange("(nt p) d -> p nt d", p=128), in_=obig
                )

            # software pipeline: loads -> scores one batch ahead of PV
            emit_loads(0)
            emit_ed_loads()
            for b in range(BPC):
                if b + 1 < BPC:
                    emit_loads(b + 1)
                emit_scores(b)
                if b > 0:
                    emit_pv(b - 1)
            emit_pv(BPC - 1)

    nc.compile()
    return nc


_NC = None


def _get_module():
    global _NC
    if _NC is None:
        _NC = _build_module()
    return _NC


def prepare_inputs(x, mask):
    """Host-side prep: per-core input dicts (cheap O(B*T*D) / O(T^2) work)."""
    x = np.asarray(x, dtype=np.float32)
    m = np.asarray(mask).astype(np.float32)
    idx = np.arange(T)
    dist = np.minimum(idx[None, :] - idx[:, None], DIST_CAP).astype(np.float64)
    edT = np.exp(dist).astype(np.float16)  # [s, t]
    ed = np.ascontiguousarray(edT.reshape(NT, 128, T))
    xm_full = np.concatenate([x * m[:, :, None], m[:, :, None]], axis=2).astype(np.float16)
    xT_full = np.ascontiguousarray(x.transpose(0, 2, 1))
    mt_full = np.ascontiguousarray(m.reshape(B // BPC, BPC, NT, 128))
    in_maps = []
    for c in range(NCORES):
        sl = slice(c * BPC, (c + 1) * BPC)
        in_maps.append(
            {
                "xT": np.ascontiguousarray(xT_full[sl]),
                "xm": np.ascontiguousarray(xm_full[sl]),
                "ed": ed,
                "mt": mt_full[c],
            }
        )
    return in_maps


def kernel(x, mask):
    nc = _get_module()
    in_maps = prepare_inputs(x, mask)
    res = run_bass_kernel_spmd(nc, in_maps, core_ids=list(range(NCORES)))
    out = np.concatenate([res.results[c]["y"] for c in range(NCORES)], axis=0)
    return out.reshape(B, T, D).astype(np.float32)


# revision 16
# speedup vs baseline: 47372.0422x; 33207.3299x over previous
"""IntraSentenceAttention Trainium2 kernel.

out[b,t,:] = sum_s P[t,s] x[b,s,:],  P = softmax-like row-normalized
             exp(x@x^T + min(t-s,10)) masked by m_t, m_s  (no max-subtract).

Strategy (8 NeuronCores, data-parallel over batch, 4 batches/core):
  - scores computed transposed, E^T[s,t], so the PV matmul needs no transpose:
      S^T tile = matmul(lhsT=xT[:,s_blk], rhs=xT[:,t_blk])   (f32r, full rate)
      E^T      = exp(S^T - 2) * expdistT[s,t]                (ACT exp + DVE mul)
    The -2 bias keeps values in fp16 range (max exp(10+score) ~ 9.9e4 > fp16max);
    a constant factor cancels in the row normalization.
  - dist bias applied multiplicatively: expdistT = exp(min(t-s,10)) precomputed
    host-side (fp16, resident in SBUF). Tiles with t_tile <= s_tile-2 are exactly
    zero (fp32/fp16 underflow) and are skipped everywhere: 43 of 64 tiles survive.
  - mask folded host-side into the PV moving operand: xm = [m_s*x | m_s] fp16.
    Column 128 of the PV accumulation then yields the row sum r_t for free.
  - epilogue per 128-row output tile: out = (acc * 1/(r+eps') * m_t), eps' = eps*e^-2.
"""

import sys

sys.path.insert(0, "/opt/trn_rl_repo")

import numpy as np

import concourse.bacc as bacc
import concourse.tile as tile
from concourse import mybir
from concourse.bass_utils import run_bass_kernel_spmd

B, T, D = 32, 1024, 128
NCORES = 8
BPC = B // NCORES          # batches per core
NT = T // 128              # 128-row tiles per sequence
DIST_CAP = 10
EXP_BIAS = -2.0            # folded into exp; cancels in normalization
EPS = 1e-7 * float(np.exp(EXP_BIAS))

F32 = mybir.dt.float32
F32R = mybir.dt.float32r
F16 = mybir.dt.float16


def _span(s):
    """Nonzero t-range start for s-tile row: tiles with t_tile <= s_tile-2 are 0."""
    return max(0, s - 1) * 128


def _chunks(w):
    """Split a score-row width into matmul chunks that never cross a PSUM bank
    (512 f32 cols) boundary. All chunks >=256 run f32r at full rate; the one
    128-wide tail (w==640) pays the 4x f32 rate on 128 cols only."""
    if w <= 512:
        return [w]
    if w == 640:
        return [512, 128]
    return [512, w - 512]


def _build_module():
    nc = bacc.Bacc("TRN2", target_bir_lowering=False, debug=False, num_devices=NCORES)
    xT = nc.declare_dram_parameter("xT", [BPC, D, T], F32R, isOutput=False)
    xm = nc.declare_dram_parameter("xm", [BPC, T, D + 1], F16, isOutput=False)
    ed = nc.declare_dram_parameter("ed", [NT, 128, T], F16, isOutput=False)
    mt = nc.declare_dram_parameter("mt", [BPC, NT, 128], F32, isOutput=False)
    y = nc.declare_dram_parameter("y", [BPC, T, D], F32, isOutput=True)

    Exp = mybir.ActivationFunctionType.Exp
    Mult = mybir.AluOpType.mult

    with tile.TileContext(nc) as tc:
        with (
            tc.tile_pool(name="edp", bufs=1) as edp,
            tc.tile_pool(name="xtp", bufs=2) as xtp,
            tc.tile_pool(name="xmp", bufs=2) as xmp,
            tc.tile_pool(name="ep", bufs=2) as epool,
            tc.tile_pool(name="sm", bufs=8) as small,
            tc.tile_pool(name="op", bufs=4) as opool,
            tc.tile_pool(name="ps_s", bufs=3, space="PSUM") as ps_s,
            tc.tile_pool(name="ps_o", bufs=2, space="PSUM") as ps_o,
        ):
            bias_t = edp.tile([128, 1], F32, tag="bias")
            nc.vector.memset(bias_t, EXP_BIAS)

            # resident distance-bias tiles: one tile, loaded per s-row so the
            # first rows are usable before the whole 2MB lands
            edt = edp.tile([128, NT, T], F16, tag="ed")
            ed_tiles = [edt[:, s, _span(s) : T] for s in range(NT)]

            def emit_ed_loads():
                for s in range(NT):
                    t0 = _span(s)
                    nc.sync.dma_start(out=edt[:, s, t0:T], in_=ed[s, :, t0:T])

            state = {}
            loads = {}

            def emit_loads(b):
                xTt = xtp.tile([128, T], F32R, tag="xT")
                nc.sync.dma_start(out=xTt, in_=xT[b])
                # one strided DMA each for the PV moving operand and the t-mask
                xmt = xmp.tile([128, NT, D + 1], F16, tag="xm")
                nc.sync.dma_start(
                    out=xmt, in_=xm[b].rearrange("(nt p) c -> p nt c", p=128)
                )
                mtile = xmp.tile([128, NT], F32, tag="mtile")
                nc.sync.dma_start(out=mtile, in_=mt[b].rearrange("nt p -> p nt"))
                loads[b] = (xTt, xmt, mtile)

            def emit_score_row(b, s):
                xTt, xmt, mtile = loads[b]
                t0 = _span(s)
                w = T - t0
                pS = ps_s.tile([128, w], F32, tag="pscore")
                off = 0
                for cw in _chunks(w):
                    nc.tensor.matmul(
                        pS[:, off : off + cw],
                        lhsT=xTt[:, s * 128 : (s + 1) * 128],
                        rhs=xTt[:, t0 + off : t0 + off + cw],
                        start=True,
                        stop=True,
                    )
                    off += cw
                Et = epool.tile([128, w], F16, tag=f"E{s}")
                nc.scalar.activation(out=Et, in_=pS, func=Exp, bias=bias_t, scale=1.0)
                nc.vector.tensor_mul(Et, Et, ed_tiles[s])
                if s == 0:
                    state[b] = (xmt, mtile, [])
                state[b][2].append(Et)

            def emit_pv_tile(b, tt):
                xmt, mtile, Es = state[b][:3]
                if tt == 0:
                    obig_new = opool.tile([128, NT, D], F32, tag="out")
                    state[b] = (xmt, mtile, Es, obig_new)
                obig = state[b][3]
                pO = ps_o.tile([128, D + 1], F32, tag="pout")
                s_list = list(range(min(NT, tt + 2)))
                for i, s in enumerate(s_list):
                    sl = tt * 128 - _span(s)
                    nc.tensor.matmul(
                        pO,
                        lhsT=Es[s][:, sl : sl + 128],
                        rhs=xmt[:, s, :],
                        start=(i == 0),
                        stop=(i == len(s_list) - 1),
                    )
                rr = small.tile([128, 1], F32, tag="r")
                nc.vector.tensor_scalar_add(rr, pO[:, D : D + 1], EPS)
                nc.vector.reciprocal(rr, rr)
                nc.vector.tensor_scalar(
                    out=obig[:, tt, :],
                    in0=pO[:, 0:D],
                    scalar1=rr,
                    scalar2=mtile[:, tt : tt + 1],
                    op0=Mult,
                    op1=Mult,
                )
                if tt == NT - 1:
                    nc.sync.dma_start(
                        out=y[b].rearrange("(nt p) d -> p nt d", p=128), in_=obig
                    )
                    state.pop(b)
                    loads.pop(b)

            # software pipeline: loads one batch ahead; PV tiles of batch b-1
            # interleaved between score rows of batch b so PE and ACT stay
            # co-busy through the tail
            emit_loads(0)
            emit_ed_loads()
            for b in range(BPC):
                if b + 1 < BPC:
                    emit_loads(b + 1)
                for s in range(NT):
                    emit_score_row(b, s)
                    if b > 0:
                        emit_pv_tile(b - 1, s)
            for tt in range(NT):
                emit_pv_tile(BPC - 1, tt)

    nc.compile()
    return nc


_NC = None


def _get_module():
    global _NC
    if _NC is None:
        _NC = _build_module()
    return _NC


def prepare_inputs(x, mask):
    """Host-side prep: per-core input dicts (cheap O(B*T*D) / O(T^2) work)."""
    x = np.asarray(x, dtype=np.float32)
    m = np.asarray(mask).astype(np.float32)
    idx = np.arange(T)
    dist = np.minimum(idx[None, :] - idx[:, None], DIST_CAP).astype(np.float64)
    edT = np.exp(dist).astype(np.float16)  # [s, t]
    ed = np.ascontiguousarray(edT.reshape(NT, 128, T))
    xm_full = np.concatenate([x * m[:, :, None], m[:, :, None]], axis=2).astype(np.float16)
    xT_full = np.ascontiguousarray(x.transpose(0, 2, 1))
    mt_full = np.ascontiguousarray(m.reshape(B // BPC, BPC, NT, 128))
    in_maps = []
    for c in range(NCORES):
        sl = slice(c * BPC, (c + 1) * BPC)
        in_maps.append(
            {
                "xT": np.ascontiguousarray(xT_full[sl]),
                "xm": np.ascontiguousarray(xm_full[sl]),
                "ed": ed,
                "mt": mt_full[c],
            }
        )
    return in_maps


def kernel(x, mask):
    nc = _get_module()
    in_maps = prepare_inputs(x, mask)
    res = run_bass_kernel_spmd(nc, in_maps, core_ids=list(range(NCORES)))
    out = np.concatenate([res.results[c]["y"] for c in range(NCORES)], axis=0)
    return out.reshape(B, T, D).astype(np.float32)
